# revision 8
# baseline (speedup 1.0000x reference)
"""Grouped expert MLP (SwiGLU MoE, 64 experts) on 8 Trainium2 NeuronCores.

Sharding: expert-parallel. Core c owns experts [8c, 8c+8) and their token
segments (32 tokens each, contiguous by construction).

The problem is HBM-bandwidth bound: each core streams its 8 experts'
weights (w1/w3/w2) exactly once. All matmul operands are cast to fp16 on
the host, halving HBM traffic vs fp32 (fp16's 10 mantissa bits keep the
relative error ~1e-3, well inside the 2e-2 gate).

Device-side layout: weights are pre-permuted on the host so every weight
DMA is one fully contiguous block per partition (8 KB lines) and the
contraction axis lands on SBUF partitions with no on-chip weight
transposes. Each expert's weights arrive as SIX 1 MiB DMAs (w1/w3/w2 x
two 512-col halves) so PE work trickles in every ~3 us; idle gaps stay
under the ~3.4 us HAM MID window and the PE keeps its 2.4 GHz clock
(a single 6 MiB DMA would leave >3.4 us PE-idle gaps, re-throttling the
PE to 1.2 GHz and making it the bottleneck).

Per expert e (d = p*KT + k for w1/w3/x, f = k*128 + p for w2/hT):
  gate[t,f] += XT[d,t].T @ W1T[d,f]   (lhsT = x slice, moving = weight, N=512)
  h = fp16(silu(gate) * up)
  hT = PE-transpose(h)                 (8 x [32,128] -> [128,32])
  y[t,d] += hT[f,t].T @ W2T[f,d]
"""

import os
from contextlib import ExitStack

import numpy as np

import concourse.bass as bass
import concourse.tile as tile
from concourse import mybir
from concourse.bass_utils import run_bass_kernel_spmd

E, T, D, F = 64, 2048, 1024, 1024
SEG = T // E           # 32 tokens per expert
N_CORES = 8
EPC = E // N_CORES     # 8 experts per core
TPC = T // N_CORES     # 256 tokens per core
KT = D // 128          # 8 contraction tiles of 128
FB = 512               # moving free-dim block (one PSUM bank of fp32)

F32 = mybir.dt.float32
F16 = mybir.dt.float16
NPF16 = np.float16


def _pe_absorb(nc, *aps):
    """Standalone 1x2 LDWEIGHTS on the PE queue that 'read' the given tiles.

    Matmult lowers through an LDWEIGHTS struct with a single sync-wait
    slot; a real matmul whose operands need 2+ semaphore waits fails
    walrus codegen ("Too many sync wait commands"). These dummy weight
    loads each absorb one dependency into the PE engine's observed
    vector clock so the real matmuls that follow need no waits. No PSUM
    write, so no bank-WAW self-sems either.
    """
    for ap in aps:
        nc.tensor.ldweights(ap.bitcast(mybir.dt.bfloat16))


def build_bass():
    nc = bass.Bass(trn_type="TRN2")

    xt = nc.dram_tensor("xt", (128, KT, TPC), F16, kind="ExternalInput")
    # per expert, per matrix, two 512-wide column halves; host-packed so
    # each (e, half) is contiguous: shape (EPC, 2, 128, KT, FB)
    w1t = nc.dram_tensor("w1t", (EPC, 2, 128, KT, FB), F16, kind="ExternalInput")
    w3t = nc.dram_tensor("w3t", (EPC, 2, 128, KT, FB), F16, kind="ExternalInput")
    w2t = nc.dram_tensor("w2t", (EPC, 2, 128, KT, FB), F16, kind="ExternalInput")
    ident = nc.dram_tensor("ident", (SEG, SEG), F16, kind="ExternalInput")
    y = nc.dram_tensor("y", (TPC, D), F32, kind="ExternalOutput")

    with ExitStack() as ctx:
        tc = ctx.enter_context(tile.TileContext(nc))
        const = ctx.enter_context(tc.tile_pool(name="const", bufs=1))
        xpool = ctx.enter_context(tc.tile_pool(name="xpool", bufs=1))
        # 6 slots of [128, KT, FB] fp16 (8 KB/partition): one expert of
        # lookahead at 6 tiles per expert. Slot-reuse distance MUST stay
        # 1 expert: the slot-WAR PE wait's publisher then postdates the
        # slot's lane-FIFO predecessor, so the strip pass can drop the
        # lane wait and every DMA keeps a single sync-wait slot.
        wpool = ctx.enter_context(tc.tile_pool(name="wpool", bufs=6))
        # rotation >= live window for every small tile: a slot is never
        # reused while any dependency on its previous tenant could still
        # force a (wait-slot-limited) semaphore wait
        spool = ctx.enter_context(tc.tile_pool(name="spool", bufs=EPC + 1))
        dpool = ctx.enter_context(tc.tile_pool(name="dpool", bufs=EPC + 1))
        psg = ctx.enter_context(tc.tile_pool(name="psg", bufs=1, space="PSUM"))
        psu = ctx.enter_context(tc.tile_pool(name="psu", bufs=1, space="PSUM"))
        psy = ctx.enter_context(tc.tile_pool(name="psy", bufs=1, space="PSUM"))
        psh = ctx.enter_context(tc.tile_pool(name="psh", bufs=2, space="PSUM"))

        id_t = const.tile([SEG, SEG], F16)
        # tiny (2KB): issued first so it eats the DMA-ring cold-start, and
        # its lane value enters the PE clock before any weight-slot reuse
        nc.sync.dma_start(id_t[:], ident[:])
        # Whole x shard resident: [128, KT, TPC]; d = p*KT + k
        XT = xpool.tile([128, KT, TPC], F16)

        for e in range(EPC):
            ts = slice(e * SEG, (e + 1) * SEG)

            wt = []  # w1a, w1b, w3a, w3b, w2a, w2b
            for i, (src, half) in enumerate(
                ((w1t, 0), (w1t, 1), (w3t, 0), (w3t, 1), (w2t, 0), (w2t, 1))
            ):
                w = wpool.tile([128, KT, FB], F16, tag="w")
                nc.sync.dma_start(w[:], src[e, half])
                wt.append(w)
                # ramp: slip x onto the ring BEHIND the first weight chunk
                # so the first gate matmul isn't delayed by it
                if e == 0 and i == 0:
                    nc.sync.dma_start(XT[:], xt[:])
            w1a, w1b, w3a, w3b, w2a, w2b = wt

            if e == 0:
                _pe_absorb(nc, id_t[:1, :1], XT[:1, 0, :1])
            _pe_absorb(
                nc, w1a[:1, 0, :1], w1b[:1, 0, :1], w3a[:1, 0, :1], w3b[:1, 0, :1]
            )
            g_ps = psg.tile([SEG, F], F32, tag="g")
            u_ps = psu.tile([SEG, F], F32, tag="u")
            for fb, w in ((0, w1a), (1, w1b)):
                fs = slice(fb * FB, (fb + 1) * FB)
                for k in range(KT):
                    nc.tensor.matmul(
                        g_ps[:, fs],
                        XT[:, k, ts],
                        w[:, k, :],
                        start=(k == 0),
                        stop=(k == KT - 1),
                    )
            for fb, w in ((0, w3a), (1, w3b)):
                fs = slice(fb * FB, (fb + 1) * FB)
                for k in range(KT):
                    nc.tensor.matmul(
                        u_ps[:, fs],
                        XT[:, k, ts],
                        w[:, k, :],
                        start=(k == 0),
                        stop=(k == KT - 1),
                    )

            # h = fp16(silu(gate) * up), then hT = PE-transpose(h), all in
            # 512-col halves so the transposes finish right behind the up
            # matmuls and the down matmuls can start the moment w2 lands
            s_sb = spool.tile([SEG, F], F32, tag="s")
            h_sb = spool.tile([SEG, F], F16, tag="h")
            ht_ps = psh.tile([128, F // 128, SEG], F16, tag="ht")
            ht_sb = spool.tile([128, F // 128, SEG], F16, tag="hts")
            for fb in range(2):
                fs = slice(fb * FB, (fb + 1) * FB)
                dust_a = dpool.tile([1, 1], F32, tag="da")
                nc.scalar.copy(dust_a[:], g_ps[:1, fs][:, :1])  # ACT absorbs PE
                nc.scalar.activation(
                    s_sb[:, fs], g_ps[:, fs], mybir.ActivationFunctionType.Silu
                )
                dust_v = dpool.tile([1, 1], F32, tag="dv")
                nc.vector.tensor_copy(dust_v[:], s_sb[:1, fs][:, :1])  # DVE<-ACT
                dust_v2 = dpool.tile([1, 1], F32, tag="dv2")
                nc.vector.tensor_copy(dust_v2[:], u_ps[:1, fs][:, :1])  # DVE<-PE
                nc.vector.tensor_mul(h_sb[:, fs], s_sb[:, fs], u_ps[:, fs])

                _pe_absorb(nc, h_sb[:1, fs][:, :1])
                ks = slice(4 * fb, 4 * fb + 4)
                for k in range(4 * fb, 4 * fb + 4):
                    nc.tensor.transpose(
                        ht_ps[:, k, :], h_sb[:, k * 128 : (k + 1) * 128], id_t[:]
                    )
                dust_h = dpool.tile([1, 1], F16, tag="dh")
                nc.scalar.copy(dust_h[:], ht_ps[:1, 4 * fb, :1])  # ACT absorbs PE
                nc.scalar.copy(ht_sb[:, ks, :], ht_ps[:, ks, :])

            _pe_absorb(
                nc, w2a[:1, 0, :1], w2b[:1, 0, :1], ht_sb[:1, F // 128 - 1, :1]
            )
            y_ps = psy.tile([SEG, D], F32, tag="y")
            y_sb = spool.tile([SEG, D], F32, tag="ysb")
            for db, w in ((0, w2a), (1, w2b)):
                ds = slice(db * FB, (db + 1) * FB)
                for k in range(F // 128):
                    nc.tensor.matmul(
                        y_ps[:, ds],
                        ht_sb[:, k, :],
                        w[:, k, :],
                        start=(k == 0),
                        stop=(k == F // 128 - 1),
                    )
                dust_a2 = dpool.tile([1, 1], F32, tag="da2")
                nc.scalar.copy(dust_a2[:], y_ps[:1, ds][:, :1])  # ACT absorbs PE
                nc.scalar.copy(y_sb[:, ds], y_ps[:, ds])
            # output DMA on the ACT HWDGE ring so it never queues behind
            # the big weight loads on the sync ring
            nc.scalar.dma_start(y[ts, :], y_sb[:])

            # completion witness: read back 4B of the rows just written and
            # consume on ACT, so the output-DMA completion enters the
            # engine-visible clock (lets the kernel-tail drain collapse to
            # a single wait; every instruction has one sync-wait slot)
            wit = dpool.tile([1, 1], F32, tag="wit")
            nc.scalar.dma_start(wit[:], y[e * SEG : e * SEG + 1, :1])
            wit_a = dpool.tile([1, 1], F32, tag="wita")
            nc.scalar.copy(wit_a[:], wit[:])

        _pe_absorb(nc, wit_a[:])

    _strip_redundant_waits(nc)
    return nc


def _strip_redundant_waits(nc):
    """Transitive (vector-clock) reduction of semaphore waits.

    Tile emits per-proc-minimal waits but not cross-proc-transitively
    minimal ones, and every TRN2 instruction struct has a single sync-wait
    slot. This pass replays the schedule abstractly, tracking each proc's
    observed semaphore clock transitively through the waits it keeps, and
    drops any wait already implied. Engine semaphores (hardware FIFO
    queues) serve as implication sources; DMA-lane sems are only ever
    dropped. Deadlock in the replay would mean an unsound drop and raises.
    """
    insts = [
        i
        for i in nc.inst_map.values()
        if i.bass_scheduled_proc is not None and i.bass_scheduled_tick is not None
    ]
    by_proc = {}
    for i in insts:
        by_proc.setdefault(i.bass_scheduled_proc, []).append(i)
    for lst in by_proc.values():
        lst.sort(key=lambda i: i.bass_scheduled_tick)

    # sem id -> single updating proc (sems with multiple updaters are never
    # used as sources and their snapshots are merged conservatively)
    upd_procs = {}
    sem_names = {}
    for i in insts:
        si = i.sync_info
        if si is None:
            continue
        for u in si.on_update:
            upd_procs.setdefault(u.id, set()).add(i.bass_scheduled_proc)
            sem_names[u.id] = u.ant_name

    engine_sems = {
        s
        for s, n in sem_names.items()
        if n.split("_")[0] in ("PE", "Activation", "DVE", "SP", "Pool")
        and len(upd_procs[s]) == 1
    }

    counters = {}
    snapshots = {}  # sem -> list of (cum_after, publisher_vc)
    vcs = {p: {} for p in by_proc}
    ptr = {p: 0 for p in by_proc}

    def merged_snapshot_vc(sem, val):
        out = {}
        for cum, svc in snapshots.get(sem, ()):
            for k, v in svc.items():
                if out.get(k, -1) < v:
                    out[k] = v
            if cum >= val:
                break
        return out

    def implied(vc, sem, val):
        return vc.get(sem, -1) >= val

    progress = True
    n_done = 0
    total = len(insts)
    while n_done < total:
        progress = False
        for p, lst in by_proc.items():
            while ptr[p] < len(lst):
                x = lst[ptr[p]]
                si = x.sync_info
                waits = list(si.on_wait) if si is not None else []
                # only imm sem-ge waits participate; others always block/keep
                ok = all(
                    counters.get(w.id, 0) >= w.wait_value
                    for w in waits
                    if w.wait_mode == "sem-ge-imm" and w.wait_value is not None
                )
                if not ok:
                    break
                vc = vcs[p]
                kept = []
                droppable = [
                    w
                    for w in waits
                    if w.wait_mode == "sem-ge-imm" and w.wait_value is not None
                ]
                fixed = [w for w in waits if w not in droppable]
                # drop waits implied by own proc clock
                droppable = [
                    w for w in droppable if not implied(vc, w.id, w.wait_value)
                ]
                # drop waits on sems this proc alone updates whose value was
                # already published by an earlier instruction of this proc:
                # engines execute their queue serially (and a DMA ring's
                # per-engine descriptor order preserves lane-sem
                # monotonicity), so FIFO order implies the wait
                droppable = [
                    w
                    for w in droppable
                    if not (
                        upd_procs.get(w.id) == {p}
                        and counters.get(w.id, 0) >= w.wait_value
                    )
                ]
                # try dropping lane (non-engine) waits implied by engine waits
                if len(droppable) + len(fixed) > 1:
                    changed = True
                    while changed and len(droppable) + len(fixed) > 1:
                        changed = False
                        for w in droppable:
                            others = [o for o in droppable if o is not w]
                            acc = dict(vc)
                            for o in others:
                                if o.id in engine_sems:
                                    for k, v in merged_snapshot_vc(
                                        o.id, o.wait_value
                                    ).items():
                                        if acc.get(k, -1) < v:
                                            acc[k] = v
                                    if acc.get(o.id, -1) < o.wait_value:
                                        acc[o.id] = o.wait_value
                            if implied(acc, w.id, w.wait_value):
                                droppable = others
                                changed = True
                                break
                kept = fixed + droppable
                # merge kept waits' knowledge into proc clock
                for w in droppable:
                    for k, v in merged_snapshot_vc(w.id, w.wait_value).items():
                        if vc.get(k, -1) < v:
                            vc[k] = v
                    if vc.get(w.id, -1) < w.wait_value:
                        vc[w.id] = w.wait_value
                if si is not None and len(kept) != len(waits):
                    x.sync_info = mybir.SyncInfo(
                        on_wait=kept, on_update=list(si.on_update)
                    )
                    si = x.sync_info
                # publish updates with current knowledge
                if si is not None:
                    for u in si.on_update:
                        if u.update_mode not in ("sem-inc", "sem-add-imm"):
                            continue
                        cum = counters.get(u.id, 0) + u.update_value
                        counters[u.id] = cum
                        snapshots.setdefault(u.id, []).append((cum, dict(vc)))
                ptr[p] += 1
                n_done += 1
                progress = True
        if not progress:
            stuck = {
                p: lst[ptr[p]].name for p, lst in by_proc.items() if ptr[p] < len(lst)
            }
            raise RuntimeError(f"wait-reduction replay deadlocked at {stuck}")

    # Kernel-tail drains/evsems have no scheduled proc; reduce their waits
    # by pairwise publisher implication (a wait is dropped when another
    # engine-sem wait's publisher had already observed it).
    for i in nc.inst_map.values():
        if i.bass_scheduled_proc is not None:
            continue
        si = i.sync_info
        if si is None or len(si.on_wait) <= 1:
            continue
        waits = [
            w
            for w in si.on_wait
            if w.wait_mode == "sem-ge-imm" and w.wait_value is not None
        ]
        fixed = [w for w in si.on_wait if w not in waits]
        changed = True
        while changed and len(waits) + len(fixed) > 1:
            changed = False
            for w in waits:
                acc = {}
                for o in waits:
                    if o is w or o.id not in engine_sems:
                        continue
                    for kk, vv in merged_snapshot_vc(o.id, o.wait_value).items():
                        if acc.get(kk, -1) < vv:
                            acc[kk] = vv
                    if acc.get(o.id, -1) < o.wait_value:
                        acc[o.id] = o.wait_value
                if implied(acc, w.id, w.wait_value):
                    waits = [o for o in waits if o is not w]
                    changed = True
                    break
        if len(waits) + len(fixed) != len(si.on_wait):
            i.sync_info = mybir.SyncInfo(
                on_wait=fixed + waits, on_update=list(si.on_update)
            )

    def _out_name(i):
        try:
            o = i.outs[0]
            t = getattr(getattr(o, "bass_ap", o), "tensor", None)
            return getattr(t, "name", None)
        except IndexError:
            return None

    # Witness read-back DMAs: drop their own-lane FIFO chain wait (the sem
    # they themselves update). Their kept RAW wait on the output DMA chains
    # them causally after every earlier same-lane DMA's consumers, and all
    # other waiters of the lane use Tile cumulative totals, so attribution
    # stays order-independent.
    for i in insts:
        si = i.sync_info
        if si is None or type(i).__name__ != "InstDMACopy":
            continue
        if _out_name(i) is None or not _out_name(i).startswith("wit"):
            continue
        own = {
            u.id
            for u in si.on_update
            if u.update_mode in ("sem-inc", "sem-add-imm")
        }
        # keep only the cross-lane RAW wait on the output DMA it reads back;
        # engine-sem waits are irrelevant to the witness's only purpose
        # (completion bookkeeping -- its value is never consumed) and its
        # own-lane FIFO wait is redundant by the totals argument above
        kept = [
            w for w in si.on_wait if w.id not in own and w.id not in engine_sems
        ]
        if len(kept) != len(si.on_wait):
            i.sync_info = mybir.SyncInfo(on_wait=kept, on_update=list(si.on_update))

    # Residual case: consecutive output DMAs chained on the same completion
    # lane. They write disjoint rows of the output tensor and nothing
    # on-device consumes them (only the kernel-tail drain waits the lane
    # total, which is order-independent: every update is +16), so the
    # lane-FIFO wait between two output DMAs is droppable.
    lane_orders = {}  # sem id -> [(cum_after, inst)]
    for p, lst in by_proc.items():
        for i in lst:
            si = i.sync_info
            if si is None or type(i).__name__ != "InstDMACopy":
                continue
            for u in si.on_update:
                if u.update_mode in ("sem-inc", "sem-add-imm"):
                    cums = lane_orders.setdefault(u.id, [])
                    prev = cums[-1][0] if cums else 0
                    cums.append((prev + u.update_value, i))
    for i in insts:
        si = i.sync_info
        if si is None or type(i).__name__ != "InstDMACopy":
            continue
        if len(si.on_wait) <= 1 or _out_name(i) != "y":
            continue
        kept = []
        for w in si.on_wait:
            pub = None
            for cum, d in lane_orders.get(w.id, ()):
                if cum >= (w.wait_value or 0):
                    pub = d
                    break
            if pub is not None and _out_name(pub) == "y":
                continue
            kept.append(w)
        if len(kept) != len(si.on_wait):
            i.sync_info = mybir.SyncInfo(on_wait=kept, on_update=list(si.on_update))


_NC_CACHE = None


def _get_nc():
    global _NC_CACHE
    if _NC_CACHE is None:
        _NC_CACHE = build_bass()
    return _NC_CACHE


def make_in_maps(x, w1, w3, w2):
    """Shard + fp16-cast + pre-permute the full inputs for the 8 cores."""
    x16 = np.asarray(x, dtype=np.float32).astype(NPF16)
    w116 = np.asarray(w1, dtype=np.float32).astype(NPF16)
    w316 = np.asarray(w3, dtype=np.float32).astype(NPF16)
    w216 = np.asarray(w2, dtype=np.float32).astype(NPF16)
    ident = np.eye(SEG, dtype=NPF16)

    in_maps = []
    for c in range(N_CORES):
        es = slice(c * EPC, (c + 1) * EPC)
        xs = x16[c * TPC : (c + 1) * TPC]  # (TPC, D)
        # xt[p, k, t] = x[t, p*KT + k]
        xt = np.ascontiguousarray(xs.reshape(TPC, 128, KT).transpose(1, 2, 0))
        # w1/w3: (EPC, F, D) -> [e, p, k, f] = w[f, p*KT + k], split f halves
        w1c = np.ascontiguousarray(
            w116[es]
            .transpose(0, 2, 1)
            .reshape(EPC, 128, KT, 2, FB)
            .transpose(0, 3, 1, 2, 4)
        )
        w3c = np.ascontiguousarray(
            w316[es]
            .transpose(0, 2, 1)
            .reshape(EPC, 128, KT, 2, FB)
            .transpose(0, 3, 1, 2, 4)
        )
        # w2: (EPC, D, F) -> [e, p, k, d] = w2[d, k*128 + p] (f = k*128+p
        # to match the PE-transpose layout of hT), split d halves
        w2c = np.ascontiguousarray(
            w216[es]
            .transpose(0, 2, 1)
            .reshape(EPC, KT, 128, 2, FB)
            .transpose(0, 3, 2, 1, 4)
        )
        in_maps.append(
            {"xt": xt, "w1t": w1c, "w3t": w3c, "w2t": w2c, "ident": ident}
        )
    return in_maps


def kernel(x, w1, w3, w2, expert_ids, seg_starts, seg_ends):
    w1 = np.asarray(w1, dtype=np.float32)
    w3 = np.asarray(w3, dtype=np.float32)
    w2 = np.asarray(w2, dtype=np.float32)
    eid = np.asarray(expert_ids).astype(np.int64)

    # reference: segment s (tokens [s*SEG, (s+1)*SEG)) uses expert_ids[s]
    if not np.array_equal(eid, np.arange(E)):
        w1, w3, w2 = w1[eid], w3[eid], w2[eid]

    in_maps = make_in_maps(x, w1, w3, w2)
    nc = _get_nc()
    res = run_bass_kernel_spmd(nc, in_maps, core_ids=list(range(N_CORES)))
    out = np.concatenate([r["y"] for r in res.results], axis=0)
    return out.astype(np.float32)


# revision 9
# speedup vs baseline: 1.1007x; 1.1007x over previous
"""Grouped expert MLP (SwiGLU MoE, 64 experts) on 8 Trainium2 NeuronCores.

Sharding: expert-parallel. Core c owns experts [8c, 8c+8) and their token
segments (32 tokens each, contiguous by construction).

The problem is HBM-bandwidth bound: each core streams its 8 experts'
weights (w1/w3/w2) exactly once. All matmul operands are cast to fp16 on
the host, halving HBM traffic vs fp32 (fp16's 10 mantissa bits keep the
relative error ~1e-3, well inside the 2e-2 gate).

Device-side layout: weights are pre-permuted on the host so every weight
DMA is one fully contiguous block per partition (8 KB lines) and the
contraction axis lands on SBUF partitions with no on-chip weight
transposes. Each expert's weights arrive as SIX 1 MiB DMAs (w1/w3/w2 x
two 512-col halves) so PE work trickles in every ~3 us; idle gaps stay
under the ~3.4 us HAM MID window and the PE keeps its 2.4 GHz clock
(a single 6 MiB DMA would leave >3.4 us PE-idle gaps, re-throttling the
PE to 1.2 GHz and making it the bottleneck).

Per expert e (d = p*KT + k for w1/w3/x, f = k*128 + p for w2/hT):
  gate[t,f] += XT[d,t].T @ W1T[d,f]   (lhsT = x slice, moving = weight, N=512)
  h = fp16(silu(gate) * up)
  hT = PE-transpose(h)                 (8 x [32,128] -> [128,32])
  y[t,d] += hT[f,t].T @ W2T[f,d]
"""

import os
from contextlib import ExitStack

import numpy as np

import concourse.bass as bass
import concourse.tile as tile
from concourse import mybir
from concourse.bass_utils import run_bass_kernel_spmd

E, T, D, F = 64, 2048, 1024, 1024
SEG = T // E           # 32 tokens per expert
N_CORES = 8
EPC = E // N_CORES     # 8 experts per core
TPC = T // N_CORES     # 256 tokens per core
KT = D // 128          # 8 contraction tiles of 128
FB = 512               # moving free-dim block (one PSUM bank of fp32)

F32 = mybir.dt.float32
F16 = mybir.dt.float16
NPF16 = np.float16


def _pe_absorb(nc, *aps):
    """Standalone 1x2 LDWEIGHTS on the PE queue that 'read' the given tiles.

    Matmult lowers through an LDWEIGHTS struct with a single sync-wait
    slot; a real matmul whose operands need 2+ semaphore waits fails
    walrus codegen ("Too many sync wait commands"). These dummy weight
    loads each absorb one dependency into the PE engine's observed
    vector clock so the real matmuls that follow need no waits. No PSUM
    write, so no bank-WAW self-sems either.
    """
    for ap in aps:
        nc.tensor.ldweights(ap.bitcast(mybir.dt.bfloat16))


def build_bass():
    nc = bass.Bass(trn_type="TRN2")

    xt = nc.dram_tensor("xt", (128, KT, TPC), F16, kind="ExternalInput")
    # per expert, per matrix, two 512-wide column halves; host-packed so
    # each (e, half) is contiguous: shape (EPC, 2, 128, KT, FB)
    w1t = nc.dram_tensor("w1t", (EPC, 2, 128, KT, FB), F16, kind="ExternalInput")
    w3t = nc.dram_tensor("w3t", (EPC, 2, 128, KT, FB), F16, kind="ExternalInput")
    w2t = nc.dram_tensor("w2t", (EPC, 2, 128, KT, FB), F16, kind="ExternalInput")
    ident = nc.dram_tensor("ident", (SEG, SEG), F16, kind="ExternalInput")
    y = nc.dram_tensor("y", (TPC, D), F32, kind="ExternalOutput")

    with ExitStack() as ctx:
        tc = ctx.enter_context(tile.TileContext(nc))
        const = ctx.enter_context(tc.tile_pool(name="const", bufs=1))
        xpool = ctx.enter_context(tc.tile_pool(name="xpool", bufs=1))
        # 6 slots of [128, KT, FB] fp16 (8 KB/partition): one expert of
        # lookahead at 6 tiles per expert. Slot-reuse distance MUST stay
        # 1 expert: the slot-WAR PE wait's publisher then postdates the
        # slot's lane-FIFO predecessor, so the strip pass can drop the
        # lane wait and every DMA keeps a single sync-wait slot.
        wpool = ctx.enter_context(tc.tile_pool(name="wpool", bufs=6))
        # rotation >= live window for every small tile: a slot is never
        # reused while any dependency on its previous tenant could still
        # force a (wait-slot-limited) semaphore wait
        spool = ctx.enter_context(tc.tile_pool(name="spool", bufs=EPC + 1))
        dpool = ctx.enter_context(tc.tile_pool(name="dpool", bufs=EPC + 1))
        psg = ctx.enter_context(tc.tile_pool(name="psg", bufs=1, space="PSUM"))
        psu = ctx.enter_context(tc.tile_pool(name="psu", bufs=1, space="PSUM"))
        psy = ctx.enter_context(tc.tile_pool(name="psy", bufs=1, space="PSUM"))
        psh = ctx.enter_context(tc.tile_pool(name="psh", bufs=2, space="PSUM"))

        id_t = const.tile([SEG, SEG], F16)
        # Whole x shard resident: [128, KT, TPC]; d = p*KT + k
        XT = xpool.tile([128, KT, TPC], F16)

        for e in range(EPC):
            ts = slice(e * SEG, (e + 1) * SEG)

            wt = []  # w1a, w1b, w3a, w3b, w2a, w2b
            for i, (src, half) in enumerate(
                ((w1t, 0), (w1t, 1), (w3t, 0), (w3t, 1), (w2t, 0), (w2t, 1))
            ):
                w = wpool.tile([128, KT, FB], F16, tag="w")
                nc.sync.dma_start(w[:], src[e, half])
                wt.append(w)
                # ramp: the ring's first 2-3 DMAs each pay a ~2.5 us
                # cold-start turnaround, so lead with the first weight
                # chunk and slip x/ident in behind it
                if e == 0 and i == 0:
                    nc.sync.dma_start(XT[:], xt[:])
                    nc.sync.dma_start(id_t[:], ident[:])
            w1a, w1b, w3a, w3b, w2a, w2b = wt

            if e == 0:
                _pe_absorb(nc, XT[:1, 0, :1], id_t[:1, :1])
            _pe_absorb(
                nc, w1a[:1, 0, :1], w1b[:1, 0, :1], w3a[:1, 0, :1], w3b[:1, 0, :1]
            )
            g_ps = psg.tile([SEG, F], F32, tag="g")
            u_ps = psu.tile([SEG, F], F32, tag="u")
            for fb, w in ((0, w1a), (1, w1b)):
                fs = slice(fb * FB, (fb + 1) * FB)
                for k in range(KT):
                    nc.tensor.matmul(
                        g_ps[:, fs],
                        XT[:, k, ts],
                        w[:, k, :],
                        start=(k == 0),
                        stop=(k == KT - 1),
                    )
            for fb, w in ((0, w3a), (1, w3b)):
                fs = slice(fb * FB, (fb + 1) * FB)
                for k in range(KT):
                    nc.tensor.matmul(
                        u_ps[:, fs],
                        XT[:, k, ts],
                        w[:, k, :],
                        start=(k == 0),
                        stop=(k == KT - 1),
                    )

            # h = fp16(silu(gate) * up), then hT = PE-transpose(h).
            # Experts 0..6 run silu/mul over the full F at once: the
            # resulting ~2.4 us ACT->DVE chain after the up matmuls splits
            # the PE's structural idle (ring period 19 us vs ~12 us of PE
            # work per expert) into sub-3.4 us pieces, keeping the HAM
            # clock gate at 8/8. Chunking this pipeline 512-cols-at-a-time
            # everywhere consolidates the idle into one ~6 us gap per
            # expert and re-throttles the PE every expert (measured +30 us
            # end-to-end). The LAST expert has no successor to pace, so
            # chunk it to drain the tail ~3 us sooner.
            s_sb = spool.tile([SEG, F], F32, tag="s")
            h_sb = spool.tile([SEG, F], F16, tag="h")
            ht_ps = psh.tile([128, F // 128, SEG], F16, tag="ht")
            ht_sb = spool.tile([128, F // 128, SEG], F16, tag="hts")
            fb_chunks = ((0, 1),) if e < EPC - 1 else ((0,), (1,))
            for chunk in fb_chunks:
                lo = chunk[0] * FB
                hi = (chunk[-1] + 1) * FB
                fs = slice(lo, hi)
                dust_a = dpool.tile([1, 1], F32, tag="da")
                nc.scalar.copy(dust_a[:], g_ps[:1, lo : lo + 1])  # ACT absorbs PE
                nc.scalar.activation(
                    s_sb[:, fs], g_ps[:, fs], mybir.ActivationFunctionType.Silu
                )
                dust_v = dpool.tile([1, 1], F32, tag="dv")
                nc.vector.tensor_copy(dust_v[:], s_sb[:1, lo : lo + 1])  # DVE<-ACT
                dust_v2 = dpool.tile([1, 1], F32, tag="dv2")
                nc.vector.tensor_copy(dust_v2[:], u_ps[:1, lo : lo + 1])  # DVE<-PE
                nc.vector.tensor_mul(h_sb[:, fs], s_sb[:, fs], u_ps[:, fs])

                _pe_absorb(nc, h_sb[:1, lo : lo + 1])
                ks = slice(lo // 128, hi // 128)
                for k in range(lo // 128, hi // 128):
                    nc.tensor.transpose(
                        ht_ps[:, k, :], h_sb[:, k * 128 : (k + 1) * 128], id_t[:]
                    )
                dust_h = dpool.tile([1, 1], F16, tag="dh")
                nc.scalar.copy(dust_h[:], ht_ps[:1, lo // 128, :1])  # ACT absorbs PE
                nc.scalar.copy(ht_sb[:, ks, :], ht_ps[:, ks, :])

            _pe_absorb(
                nc, w2a[:1, 0, :1], w2b[:1, 0, :1], ht_sb[:1, F // 128 - 1, :1]
            )
            y_ps = psy.tile([SEG, D], F32, tag="y")
            y_sb = spool.tile([SEG, D], F32, tag="ysb")
            for db, w in ((0, w2a), (1, w2b)):
                ds = slice(db * FB, (db + 1) * FB)
                for k in range(F // 128):
                    nc.tensor.matmul(
                        y_ps[:, ds],
                        ht_sb[:, k, :],
                        w[:, k, :],
                        start=(k == 0),
                        stop=(k == F // 128 - 1),
                    )
                dust_a2 = dpool.tile([1, 1], F32, tag="da2")
                nc.scalar.copy(dust_a2[:], y_ps[:1, ds][:, :1])  # ACT absorbs PE
                nc.scalar.copy(y_sb[:, ds], y_ps[:, ds])
            # output DMA on the ACT HWDGE ring so it never queues behind
            # the big weight loads on the sync ring
            nc.scalar.dma_start(y[ts, :], y_sb[:])

            # completion witness: read back 4B of the rows just written and
            # consume on ACT, so the output-DMA completion enters the
            # engine-visible clock (lets the kernel-tail drain collapse to
            # a single wait; every instruction has one sync-wait slot)
            wit = dpool.tile([1, 1], F32, tag="wit")
            nc.scalar.dma_start(wit[:], y[e * SEG : e * SEG + 1, :1])
            wit_a = dpool.tile([1, 1], F32, tag="wita")
            nc.scalar.copy(wit_a[:], wit[:])

        _pe_absorb(nc, wit_a[:])

    _strip_redundant_waits(nc)
    return nc


def _strip_redundant_waits(nc):
    """Transitive (vector-clock) reduction of semaphore waits.

    Tile emits per-proc-minimal waits but not cross-proc-transitively
    minimal ones, and every TRN2 instruction struct has a single sync-wait
    slot. This pass replays the schedule abstractly, tracking each proc's
    observed semaphore clock transitively through the waits it keeps, and
    drops any wait already implied. Engine semaphores (hardware FIFO
    queues) serve as implication sources; DMA-lane sems are only ever
    dropped. Deadlock in the replay would mean an unsound drop and raises.
    """
    insts = [
        i
        for i in nc.inst_map.values()
        if i.bass_scheduled_proc is not None and i.bass_scheduled_tick is not None
    ]
    by_proc = {}
    for i in insts:
        by_proc.setdefault(i.bass_scheduled_proc, []).append(i)
    for lst in by_proc.values():
        lst.sort(key=lambda i: i.bass_scheduled_tick)

    # sem id -> single updating proc (sems with multiple updaters are never
    # used as sources and their snapshots are merged conservatively)
    upd_procs = {}
    sem_names = {}
    for i in insts:
        si = i.sync_info
        if si is None:
            continue
        for u in si.on_update:
            upd_procs.setdefault(u.id, set()).add(i.bass_scheduled_proc)
            sem_names[u.id] = u.ant_name

    engine_sems = {
        s
        for s, n in sem_names.items()
        if n.split("_")[0] in ("PE", "Activation", "DVE", "SP", "Pool")
        and len(upd_procs[s]) == 1
    }

    counters = {}
    snapshots = {}  # sem -> list of (cum_after, publisher_vc)
    vcs = {p: {} for p in by_proc}
    ptr = {p: 0 for p in by_proc}

    def merged_snapshot_vc(sem, val):
        out = {}
        for cum, svc in snapshots.get(sem, ()):
            for k, v in svc.items():
                if out.get(k, -1) < v:
                    out[k] = v
            if cum >= val:
                break
        return out

    def implied(vc, sem, val):
        return vc.get(sem, -1) >= val

    progress = True
    n_done = 0
    total = len(insts)
    while n_done < total:
        progress = False
        for p, lst in by_proc.items():
            while ptr[p] < len(lst):
                x = lst[ptr[p]]
                si = x.sync_info
                waits = list(si.on_wait) if si is not None else []
                # only imm sem-ge waits participate; others always block/keep
                ok = all(
                    counters.get(w.id, 0) >= w.wait_value
                    for w in waits
                    if w.wait_mode == "sem-ge-imm" and w.wait_value is not None
                )
                if not ok:
                    break
                vc = vcs[p]
                kept = []
                droppable = [
                    w
                    for w in waits
                    if w.wait_mode == "sem-ge-imm" and w.wait_value is not None
                ]
                fixed = [w for w in waits if w not in droppable]
                # drop waits implied by own proc clock
                droppable = [
                    w for w in droppable if not implied(vc, w.id, w.wait_value)
                ]
                # drop waits on sems this proc alone updates whose value was
                # already published by an earlier instruction of this proc:
                # engines execute their queue serially (and a DMA ring's
                # per-engine descriptor order preserves lane-sem
                # monotonicity), so FIFO order implies the wait
                droppable = [
                    w
                    for w in droppable
                    if not (
                        upd_procs.get(w.id) == {p}
                        and counters.get(w.id, 0) >= w.wait_value
                    )
                ]
                # try dropping lane (non-engine) waits implied by engine waits
                if len(droppable) + len(fixed) > 1:
                    changed = True
                    while changed and len(droppable) + len(fixed) > 1:
                        changed = False
                        for w in droppable:
                            others = [o for o in droppable if o is not w]
                            acc = dict(vc)
                            for o in others:
                                if o.id in engine_sems:
                                    for k, v in merged_snapshot_vc(
                                        o.id, o.wait_value
                                    ).items():
                                        if acc.get(k, -1) < v:
                                            acc[k] = v
                                    if acc.get(o.id, -1) < o.wait_value:
                                        acc[o.id] = o.wait_value
                            if implied(acc, w.id, w.wait_value):
                                droppable = others
                                changed = True
                                break
                kept = fixed + droppable
                # merge kept waits' knowledge into proc clock
                for w in droppable:
                    for k, v in merged_snapshot_vc(w.id, w.wait_value).items():
                        if vc.get(k, -1) < v:
                            vc[k] = v
                    if vc.get(w.id, -1) < w.wait_value:
                        vc[w.id] = w.wait_value
                if si is not None and len(kept) != len(waits):
                    x.sync_info = mybir.SyncInfo(
                        on_wait=kept, on_update=list(si.on_update)
                    )
                    si = x.sync_info
                # publish updates with current knowledge
                if si is not None:
                    for u in si.on_update:
                        if u.update_mode not in ("sem-inc", "sem-add-imm"):
                            continue
                        cum = counters.get(u.id, 0) + u.update_value
                        counters[u.id] = cum
                        snapshots.setdefault(u.id, []).append((cum, dict(vc)))
                ptr[p] += 1
                n_done += 1
                progress = True
        if not progress:
            stuck = {
                p: lst[ptr[p]].name for p, lst in by_proc.items() if ptr[p] < len(lst)
            }
            raise RuntimeError(f"wait-reduction replay deadlocked at {stuck}")

    # Kernel-tail drains/evsems have no scheduled proc; reduce their waits
    # by pairwise publisher implication (a wait is dropped when another
    # engine-sem wait's publisher had already observed it).
    for i in nc.inst_map.values():
        if i.bass_scheduled_proc is not None:
            continue
        si = i.sync_info
        if si is None or len(si.on_wait) <= 1:
            continue
        waits = [
            w
            for w in si.on_wait
            if w.wait_mode == "sem-ge-imm" and w.wait_value is not None
        ]
        fixed = [w for w in si.on_wait if w not in waits]
        changed = True
        while changed and len(waits) + len(fixed) > 1:
            changed = False
            for w in waits:
                acc = {}
                for o in waits:
                    if o is w or o.id not in engine_sems:
                        continue
                    for kk, vv in merged_snapshot_vc(o.id, o.wait_value).items():
                        if acc.get(kk, -1) < vv:
                            acc[kk] = vv
                    if acc.get(o.id, -1) < o.wait_value:
                        acc[o.id] = o.wait_value
                if implied(acc, w.id, w.wait_value):
                    waits = [o for o in waits if o is not w]
                    changed = True
                    break
        if len(waits) + len(fixed) != len(si.on_wait):
            i.sync_info = mybir.SyncInfo(
                on_wait=fixed + waits, on_update=list(si.on_update)
            )

    def _out_name(i):
        try:
            o = i.outs[0]
            t = getattr(getattr(o, "bass_ap", o), "tensor", None)
            return getattr(t, "name", None)
        except IndexError:
            return None

    # Witness read-back DMAs: drop their own-lane FIFO chain wait (the sem
    # they themselves update). Their kept RAW wait on the output DMA chains
    # them causally after every earlier same-lane DMA's consumers, and all
    # other waiters of the lane use Tile cumulative totals, so attribution
    # stays order-independent.
    for i in insts:
        si = i.sync_info
        if si is None or type(i).__name__ != "InstDMACopy":
            continue
        if _out_name(i) is None or not _out_name(i).startswith("wit"):
            continue
        own = {
            u.id
            for u in si.on_update
            if u.update_mode in ("sem-inc", "sem-add-imm")
        }
        # keep only the cross-lane RAW wait on the output DMA it reads back;
        # engine-sem waits are irrelevant to the witness's only purpose
        # (completion bookkeeping -- its value is never consumed) and its
        # own-lane FIFO wait is redundant by the totals argument above
        kept = [
            w for w in si.on_wait if w.id not in own and w.id not in engine_sems
        ]
        if len(kept) != len(si.on_wait):
            i.sync_info = mybir.SyncInfo(on_wait=kept, on_update=list(si.on_update))

    # Residual case: consecutive output DMAs chained on the same completion
    # lane. They write disjoint rows of the output tensor and nothing
    # on-device consumes them (only the kernel-tail drain waits the lane
    # total, which is order-independent: every update is +16), so the
    # lane-FIFO wait between two output DMAs is droppable.
    lane_orders = {}  # sem id -> [(cum_after, inst)]
    for p, lst in by_proc.items():
        for i in lst:
            si = i.sync_info
            if si is None or type(i).__name__ != "InstDMACopy":
                continue
            for u in si.on_update:
                if u.update_mode in ("sem-inc", "sem-add-imm"):
                    cums = lane_orders.setdefault(u.id, [])
                    prev = cums[-1][0] if cums else 0
                    cums.append((prev + u.update_value, i))
    for i in insts:
        si = i.sync_info
        if si is None or type(i).__name__ != "InstDMACopy":
            continue
        if len(si.on_wait) <= 1 or _out_name(i) != "y":
            continue
        kept = []
        for w in si.on_wait:
            pub = None
            for cum, d in lane_orders.get(w.id, ()):
                if cum >= (w.wait_value or 0):
                    pub = d
                    break
            if pub is not None and _out_name(pub) == "y":
                continue
            kept.append(w)
        if len(kept) != len(si.on_wait):
            i.sync_info = mybir.SyncInfo(on_wait=kept, on_update=list(si.on_update))


_NC_CACHE = None


def _get_nc():
    global _NC_CACHE
    if _NC_CACHE is None:
        _NC_CACHE = build_bass()
    return _NC_CACHE


def make_in_maps(x, w1, w3, w2):
    """Shard + fp16-cast + pre-permute the full inputs for the 8 cores."""
    x16 = np.asarray(x, dtype=np.float32).astype(NPF16)
    w116 = np.asarray(w1, dtype=np.float32).astype(NPF16)
    w316 = np.asarray(w3, dtype=np.float32).astype(NPF16)
    w216 = np.asarray(w2, dtype=np.float32).astype(NPF16)
    ident = np.eye(SEG, dtype=NPF16)

    in_maps = []
    for c in range(N_CORES):
        es = slice(c * EPC, (c + 1) * EPC)
        xs = x16[c * TPC : (c + 1) * TPC]  # (TPC, D)
        # xt[p, k, t] = x[t, p*KT + k]
        xt = np.ascontiguousarray(xs.reshape(TPC, 128, KT).transpose(1, 2, 0))
        # w1/w3: (EPC, F, D) -> [e, p, k, f] = w[f, p*KT + k], split f halves
        w1c = np.ascontiguousarray(
            w116[es]
            .transpose(0, 2, 1)
            .reshape(EPC, 128, KT, 2, FB)
            .transpose(0, 3, 1, 2, 4)
        )
        w3c = np.ascontiguousarray(
            w316[es]
            .transpose(0, 2, 1)
            .reshape(EPC, 128, KT, 2, FB)
            .transpose(0, 3, 1, 2, 4)
        )
        # w2: (EPC, D, F) -> [e, p, k, d] = w2[d, k*128 + p] (f = k*128+p
        # to match the PE-transpose layout of hT), split d halves
        w2c = np.ascontiguousarray(
            w216[es]
            .transpose(0, 2, 1)
            .reshape(EPC, KT, 128, 2, FB)
            .transpose(0, 3, 2, 1, 4)
        )
        in_maps.append(
            {"xt": xt, "w1t": w1c, "w3t": w3c, "w2t": w2c, "ident": ident}
        )
    return in_maps


def kernel(x, w1, w3, w2, expert_ids, seg_starts, seg_ends):
    w1 = np.asarray(w1, dtype=np.float32)
    w3 = np.asarray(w3, dtype=np.float32)
    w2 = np.asarray(w2, dtype=np.float32)
    eid = np.asarray(expert_ids).astype(np.int64)

    # reference: segment s (tokens [s*SEG, (s+1)*SEG)) uses expert_ids[s]
    if not np.array_equal(eid, np.arange(E)):
        w1, w3, w2 = w1[eid], w3[eid], w2[eid]

    in_maps = make_in_maps(x, w1, w3, w2)
    nc = _get_nc()
    res = run_bass_kernel_spmd(nc, in_maps, core_ids=list(range(N_CORES)))
    out = np.concatenate([r["y"] for r in res.results], axis=0)
    return out.astype(np.float32)


# revision 10
# speedup vs baseline: 1.1561x; 1.0503x over previous
"""Grouped expert MLP (SwiGLU MoE, 64 experts) on 8 Trainium2 NeuronCores.

Sharding: expert-parallel. Core c owns experts [8c, 8c+8) and their token
segments (32 tokens each, contiguous by construction).

The problem is HBM-bandwidth bound: each core streams its 8 experts'
weights (w1/w3/w2) exactly once. All matmul operands are cast to fp16 on
the host, halving HBM traffic vs fp32 (fp16's 10 mantissa bits keep the
relative error ~1e-3, well inside the 2e-2 gate).

Device-side layout: weights are pre-permuted on the host so every weight
DMA is one fully contiguous block per partition (8 KB lines) and the
contraction axis lands on SBUF partitions with no on-chip weight
transposes. Each expert's weights arrive as SIX 1 MiB DMAs (w1/w3/w2 x
two 512-col halves) so PE work trickles in every ~3 us; idle gaps stay
under the ~3.4 us HAM MID window and the PE keeps its 2.4 GHz clock
(a single 6 MiB DMA would leave >3.4 us PE-idle gaps, re-throttling the
PE to 1.2 GHz and making it the bottleneck).

Per expert e (d = p*KT + k for w1/w3/x, f = k*128 + p for w2/hT):
  gate[t,f] += XT[d,t].T @ W1T[d,f]   (lhsT = x slice, moving = weight, N=512)
  h = fp16(silu(gate) * up)
  hT = PE-transpose(h)                 (8 x [32,128] -> [128,32])
  y[t,d] += hT[f,t].T @ W2T[f,d]
"""

import os
from contextlib import ExitStack

import numpy as np

import concourse.bass as bass
import concourse.tile as tile
from concourse import mybir
from concourse.bass_utils import run_bass_kernel_spmd

E, T, D, F = 64, 2048, 1024, 1024
SEG = T // E           # 32 tokens per expert
N_CORES = 8
EPC = E // N_CORES     # 8 experts per core
TPC = T // N_CORES     # 256 tokens per core
KT = D // 128          # 8 contraction tiles of 128
FB = 512               # moving free-dim block (one PSUM bank of fp32)

F32 = mybir.dt.float32
F16 = mybir.dt.float16
NPF16 = np.float16


def _pe_absorb(nc, *aps):
    """Standalone 1x2 LDWEIGHTS on the PE queue that 'read' the given tiles.

    Matmult lowers through an LDWEIGHTS struct with a single sync-wait
    slot; a real matmul whose operands need 2+ semaphore waits fails
    walrus codegen ("Too many sync wait commands"). These dummy weight
    loads each absorb one dependency into the PE engine's observed
    vector clock so the real matmuls that follow need no waits. No PSUM
    write, so no bank-WAW self-sems either.
    """
    for ap in aps:
        nc.tensor.ldweights(ap.bitcast(mybir.dt.bfloat16))


def build_bass():
    nc = bass.Bass(trn_type="TRN2")

    xt = nc.dram_tensor("xt", (128, KT, TPC), F16, kind="ExternalInput")
    # per expert, per matrix, two 512-wide column halves; host-packed so
    # each (e, half) is contiguous: shape (EPC, 2, 128, KT, FB)
    w1t = nc.dram_tensor("w1t", (EPC, 2, 128, KT, FB), F16, kind="ExternalInput")
    w3t = nc.dram_tensor("w3t", (EPC, 2, 128, KT, FB), F16, kind="ExternalInput")
    w2t = nc.dram_tensor("w2t", (EPC, 2, 128, KT, FB), F16, kind="ExternalInput")
    ident = nc.dram_tensor("ident", (SEG, SEG), F16, kind="ExternalInput")
    y = nc.dram_tensor("y", (TPC, D), F32, kind="ExternalOutput")

    with ExitStack() as ctx:
        tc = ctx.enter_context(tile.TileContext(nc))
        const = ctx.enter_context(tc.tile_pool(name="const", bufs=1))
        xpool = ctx.enter_context(tc.tile_pool(name="xpool", bufs=1))
        # 6 slots of [128, KT, FB] fp16 (8 KB/partition): one expert of
        # lookahead at 6 tiles per expert. Slot-reuse distance MUST stay
        # 1 expert: the slot-WAR PE wait's publisher then postdates the
        # slot's lane-FIFO predecessor, so the strip pass can drop the
        # lane wait and every DMA keeps a single sync-wait slot.
        wpool = ctx.enter_context(tc.tile_pool(name="wpool", bufs=6))
        # rotation >= live window for every small tile: a slot is never
        # reused while any dependency on its previous tenant could still
        # force a (wait-slot-limited) semaphore wait
        spool = ctx.enter_context(tc.tile_pool(name="spool", bufs=EPC + 1))
        dpool = ctx.enter_context(tc.tile_pool(name="dpool", bufs=EPC + 1))
        psg = ctx.enter_context(tc.tile_pool(name="psg", bufs=1, space="PSUM"))
        psu = ctx.enter_context(tc.tile_pool(name="psu", bufs=1, space="PSUM"))
        psy = ctx.enter_context(tc.tile_pool(name="psy", bufs=1, space="PSUM"))
        psh = ctx.enter_context(tc.tile_pool(name="psh", bufs=2, space="PSUM"))

        id_t = const.tile([SEG, SEG], F16)
        # Whole x shard resident: [128, KT, TPC]; d = p*KT + k
        XT = xpool.tile([128, KT, TPC], F16)

        for e in range(EPC):
            ts = slice(e * SEG, (e + 1) * SEG)

            wt = []  # w1a, w1b, w3a, w3b, w2a, w2b
            for i, (src, half) in enumerate(
                ((w1t, 0), (w1t, 1), (w3t, 0), (w3t, 1), (w2t, 0), (w2t, 1))
            ):
                w = wpool.tile([128, KT, FB], F16, tag="w")
                nc.sync.dma_start(w[:], src[e, half])
                wt.append(w)
                # ramp: the ring's first 2-3 DMAs each pay a ~2.5 us
                # cold-start turnaround, so lead with the first weight
                # chunk and slip x/ident in behind it
                if e == 0 and i == 0:
                    nc.sync.dma_start(XT[:], xt[:])
                    nc.sync.dma_start(id_t[:], ident[:])
            w1a, w1b, w3a, w3b, w2a, w2b = wt

            if e == 0:
                _pe_absorb(nc, XT[:1, 0, :1], id_t[:1, :1])
            _pe_absorb(
                nc, w1a[:1, 0, :1], w1b[:1, 0, :1], w3a[:1, 0, :1], w3b[:1, 0, :1]
            )
            g_ps = psg.tile([SEG, F], F32, tag="g")
            u_ps = psu.tile([SEG, F], F32, tag="u")
            for fb, w in ((0, w1a), (1, w1b)):
                fs = slice(fb * FB, (fb + 1) * FB)
                for k in range(KT):
                    nc.tensor.matmul(
                        g_ps[:, fs],
                        XT[:, k, ts],
                        w[:, k, :],
                        start=(k == 0),
                        stop=(k == KT - 1),
                    )
            for fb, w in ((0, w3a), (1, w3b)):
                fs = slice(fb * FB, (fb + 1) * FB)
                for k in range(KT):
                    nc.tensor.matmul(
                        u_ps[:, fs],
                        XT[:, k, ts],
                        w[:, k, :],
                        start=(k == 0),
                        stop=(k == KT - 1),
                    )

            # h = fp16(silu(gate) * up), then hT = PE-transpose(h).
            # Experts 0..6 run silu/mul over the full F at once: the
            # resulting ~2.4 us ACT->DVE chain after the up matmuls splits
            # the PE's structural idle (ring period 19 us vs ~12 us of PE
            # work per expert) into sub-3.4 us pieces, keeping the HAM
            # clock gate at 8/8. Chunking this pipeline 512-cols-at-a-time
            # everywhere consolidates the idle into one ~6 us gap per
            # expert and re-throttles the PE every expert (measured +30 us
            # end-to-end). The LAST expert has no successor to pace, so
            # chunk it to drain the tail ~3 us sooner.
            s_sb = spool.tile([SEG, F], F32, tag="s")
            h_sb = spool.tile([SEG, F], F16, tag="h")
            ht_ps = psh.tile([128, F // 128, SEG], F16, tag="ht")
            ht_sb = spool.tile([128, F // 128, SEG], F16, tag="hts")
            fb_chunks = ((0, 1),) if e < EPC - 1 else ((0,), (1,))
            # silu + mul per chunk (one chunk = full F for e<7)
            for chunk in fb_chunks:
                lo, hi = chunk[0] * FB, (chunk[-1] + 1) * FB
                fs = slice(lo, hi)
                dust_a = dpool.tile([1, 1], F32, tag="da")
                nc.scalar.copy(dust_a[:], g_ps[:1, lo : lo + 1])  # ACT absorbs PE
                nc.scalar.activation(
                    s_sb[:, fs], g_ps[:, fs], mybir.ActivationFunctionType.Silu
                )
            for chunk in fb_chunks:
                lo, hi = chunk[0] * FB, (chunk[-1] + 1) * FB
                fs = slice(lo, hi)
                dust_v = dpool.tile([1, 1], F32, tag="dv")
                nc.vector.tensor_copy(dust_v[:], s_sb[:1, lo : lo + 1])  # DVE<-ACT
                dust_v2 = dpool.tile([1, 1], F32, tag="dv2")
                nc.vector.tensor_copy(dust_v2[:], u_ps[:1, lo : lo + 1])  # DVE<-PE
                nc.vector.tensor_mul(h_sb[:, fs], s_sb[:, fs], u_ps[:, fs])
            # PE transposes per chunk, then the PSUM->SBUF evacuations; keep
            # every ACT copy AFTER all silu emissions so the tail never
            # serializes ACT behind PE transposes
            for chunk in fb_chunks:
                lo, hi = chunk[0] * FB, (chunk[-1] + 1) * FB
                _pe_absorb(nc, h_sb[:1, lo : lo + 1])
                for k in range(lo // 128, hi // 128):
                    nc.tensor.transpose(
                        ht_ps[:, k, :], h_sb[:, k * 128 : (k + 1) * 128], id_t[:]
                    )
            for chunk in fb_chunks:
                lo, hi = chunk[0] * FB, (chunk[-1] + 1) * FB
                ks = slice(lo // 128, hi // 128)
                dust_h = dpool.tile([1, 1], F16, tag="dh")
                nc.scalar.copy(dust_h[:], ht_ps[:1, lo // 128, :1])  # ACT absorbs PE
                nc.scalar.copy(ht_sb[:, ks, :], ht_ps[:, ks, :])

            _pe_absorb(
                nc, w2a[:1, 0, :1], w2b[:1, 0, :1], ht_sb[:1, F // 128 - 1, :1]
            )
            y_ps = psy.tile([SEG, D], F32, tag="y")
            y_sb = spool.tile([SEG, D], F32, tag="ysb")
            for db, w in ((0, w2a), (1, w2b)):
                ds = slice(db * FB, (db + 1) * FB)
                for k in range(F // 128):
                    nc.tensor.matmul(
                        y_ps[:, ds],
                        ht_sb[:, k, :],
                        w[:, k, :],
                        start=(k == 0),
                        stop=(k == F // 128 - 1),
                    )
                dust_a2 = dpool.tile([1, 1], F32, tag="da2")
                nc.scalar.copy(dust_a2[:], y_ps[:1, ds][:, :1])  # ACT absorbs PE
                nc.scalar.copy(y_sb[:, ds], y_ps[:, ds])
            # output DMA on the ACT HWDGE ring so it never queues behind
            # the big weight loads on the sync ring
            nc.scalar.dma_start(y[ts, :], y_sb[:])

            # completion witness: read back 4B of the rows just written and
            # consume on ACT, so the output-DMA completion enters the
            # engine-visible clock (lets the kernel-tail drain collapse to
            # a single wait; every instruction has one sync-wait slot)
            wit = dpool.tile([1, 1], F32, tag="wit")
            nc.scalar.dma_start(wit[:], y[e * SEG : e * SEG + 1, :1])
            wit_a = dpool.tile([1, 1], F32, tag="wita")
            nc.scalar.copy(wit_a[:], wit[:])

        _pe_absorb(nc, wit_a[:])

    _strip_redundant_waits(nc)
    return nc


def _strip_redundant_waits(nc):
    """Transitive (vector-clock) reduction of semaphore waits.

    Tile emits per-proc-minimal waits but not cross-proc-transitively
    minimal ones, and every TRN2 instruction struct has a single sync-wait
    slot. This pass replays the schedule abstractly, tracking each proc's
    observed semaphore clock transitively through the waits it keeps, and
    drops any wait already implied. Engine semaphores (hardware FIFO
    queues) serve as implication sources; DMA-lane sems are only ever
    dropped. Deadlock in the replay would mean an unsound drop and raises.
    """
    insts = [
        i
        for i in nc.inst_map.values()
        if i.bass_scheduled_proc is not None and i.bass_scheduled_tick is not None
    ]
    by_proc = {}
    for i in insts:
        by_proc.setdefault(i.bass_scheduled_proc, []).append(i)
    for lst in by_proc.values():
        lst.sort(key=lambda i: i.bass_scheduled_tick)

    # sem id -> single updating proc (sems with multiple updaters are never
    # used as sources and their snapshots are merged conservatively)
    upd_procs = {}
    sem_names = {}
    for i in insts:
        si = i.sync_info
        if si is None:
            continue
        for u in si.on_update:
            upd_procs.setdefault(u.id, set()).add(i.bass_scheduled_proc)
            sem_names[u.id] = u.ant_name

    engine_sems = {
        s
        for s, n in sem_names.items()
        if n.split("_")[0] in ("PE", "Activation", "DVE", "SP", "Pool")
        and len(upd_procs[s]) == 1
    }

    counters = {}
    snapshots = {}  # sem -> list of (cum_after, publisher_vc)
    vcs = {p: {} for p in by_proc}
    ptr = {p: 0 for p in by_proc}

    def merged_snapshot_vc(sem, val):
        out = {}
        for cum, svc in snapshots.get(sem, ()):
            for k, v in svc.items():
                if out.get(k, -1) < v:
                    out[k] = v
            if cum >= val:
                break
        return out

    def implied(vc, sem, val):
        return vc.get(sem, -1) >= val

    progress = True
    n_done = 0
    total = len(insts)
    while n_done < total:
        progress = False
        for p, lst in by_proc.items():
            while ptr[p] < len(lst):
                x = lst[ptr[p]]
                si = x.sync_info
                waits = list(si.on_wait) if si is not None else []
                # only imm sem-ge waits participate; others always block/keep
                ok = all(
                    counters.get(w.id, 0) >= w.wait_value
                    for w in waits
                    if w.wait_mode == "sem-ge-imm" and w.wait_value is not None
                )
                if not ok:
                    break
                vc = vcs[p]
                kept = []
                droppable = [
                    w
                    for w in waits
                    if w.wait_mode == "sem-ge-imm" and w.wait_value is not None
                ]
                fixed = [w for w in waits if w not in droppable]
                # drop waits implied by own proc clock
                droppable = [
                    w for w in droppable if not implied(vc, w.id, w.wait_value)
                ]
                # drop waits on sems this proc alone updates whose value was
                # already published by an earlier instruction of this proc:
                # engines execute their queue serially (and a DMA ring's
                # per-engine descriptor order preserves lane-sem
                # monotonicity), so FIFO order implies the wait
                droppable = [
                    w
                    for w in droppable
                    if not (
                        upd_procs.get(w.id) == {p}
                        and counters.get(w.id, 0) >= w.wait_value
                    )
                ]
                # try dropping lane (non-engine) waits implied by engine waits
                if len(droppable) + len(fixed) > 1:
                    changed = True
                    while changed and len(droppable) + len(fixed) > 1:
                        changed = False
                        for w in droppable:
                            others = [o for o in droppable if o is not w]
                            acc = dict(vc)
                            for o in others:
                                if o.id in engine_sems:
                                    for k, v in merged_snapshot_vc(
                                        o.id, o.wait_value
                                    ).items():
                                        if acc.get(k, -1) < v:
                                            acc[k] = v
                                    if acc.get(o.id, -1) < o.wait_value:
                                        acc[o.id] = o.wait_value
                            if implied(acc, w.id, w.wait_value):
                                droppable = others
                                changed = True
                                break
                kept = fixed + droppable
                # merge kept waits' knowledge into proc clock
                for w in droppable:
                    for k, v in merged_snapshot_vc(w.id, w.wait_value).items():
                        if vc.get(k, -1) < v:
                            vc[k] = v
                    if vc.get(w.id, -1) < w.wait_value:
                        vc[w.id] = w.wait_value
                if si is not None and len(kept) != len(waits):
                    x.sync_info = mybir.SyncInfo(
                        on_wait=kept, on_update=list(si.on_update)
                    )
                    si = x.sync_info
                # publish updates with current knowledge
                if si is not None:
                    for u in si.on_update:
                        if u.update_mode not in ("sem-inc", "sem-add-imm"):
                            continue
                        cum = counters.get(u.id, 0) + u.update_value
                        counters[u.id] = cum
                        snapshots.setdefault(u.id, []).append((cum, dict(vc)))
                ptr[p] += 1
                n_done += 1
                progress = True
        if not progress:
            stuck = {
                p: lst[ptr[p]].name for p, lst in by_proc.items() if ptr[p] < len(lst)
            }
            raise RuntimeError(f"wait-reduction replay deadlocked at {stuck}")

    # Kernel-tail drains/evsems have no scheduled proc; reduce their waits
    # by pairwise publisher implication (a wait is dropped when another
    # engine-sem wait's publisher had already observed it).
    for i in nc.inst_map.values():
        if i.bass_scheduled_proc is not None:
            continue
        si = i.sync_info
        if si is None or len(si.on_wait) <= 1:
            continue
        waits = [
            w
            for w in si.on_wait
            if w.wait_mode == "sem-ge-imm" and w.wait_value is not None
        ]
        fixed = [w for w in si.on_wait if w not in waits]
        changed = True
        while changed and len(waits) + len(fixed) > 1:
            changed = False
            for w in waits:
                acc = {}
                for o in waits:
                    if o is w or o.id not in engine_sems:
                        continue
                    for kk, vv in merged_snapshot_vc(o.id, o.wait_value).items():
                        if acc.get(kk, -1) < vv:
                            acc[kk] = vv
                    if acc.get(o.id, -1) < o.wait_value:
                        acc[o.id] = o.wait_value
                if implied(acc, w.id, w.wait_value):
                    waits = [o for o in waits if o is not w]
                    changed = True
                    break
        if len(waits) + len(fixed) != len(si.on_wait):
            i.sync_info = mybir.SyncInfo(
                on_wait=fixed + waits, on_update=list(si.on_update)
            )

    def _out_name(i):
        try:
            o = i.outs[0]
            t = getattr(getattr(o, "bass_ap", o), "tensor", None)
            return getattr(t, "name", None)
        except IndexError:
            return None

    # Witness read-back DMAs: drop their own-lane FIFO chain wait (the sem
    # they themselves update). Their kept RAW wait on the output DMA chains
    # them causally after every earlier same-lane DMA's consumers, and all
    # other waiters of the lane use Tile cumulative totals, so attribution
    # stays order-independent.
    for i in insts:
        si = i.sync_info
        if si is None or type(i).__name__ != "InstDMACopy":
            continue
        if _out_name(i) is None or not _out_name(i).startswith("wit"):
            continue
        own = {
            u.id
            for u in si.on_update
            if u.update_mode in ("sem-inc", "sem-add-imm")
        }
        # keep only the cross-lane RAW wait on the output DMA it reads back;
        # engine-sem waits are irrelevant to the witness's only purpose
        # (completion bookkeeping -- its value is never consumed) and its
        # own-lane FIFO wait is redundant by the totals argument above
        kept = [
            w for w in si.on_wait if w.id not in own and w.id not in engine_sems
        ]
        if len(kept) != len(si.on_wait):
            i.sync_info = mybir.SyncInfo(on_wait=kept, on_update=list(si.on_update))

    # Residual case: consecutive output DMAs chained on the same completion
    # lane. They write disjoint rows of the output tensor and nothing
    # on-device consumes them (only the kernel-tail drain waits the lane
    # total, which is order-independent: every update is +16), so the
    # lane-FIFO wait between two output DMAs is droppable.
    lane_orders = {}  # sem id -> [(cum_after, inst)]
    for p, lst in by_proc.items():
        for i in lst:
            si = i.sync_info
            if si is None or type(i).__name__ != "InstDMACopy":
                continue
            for u in si.on_update:
                if u.update_mode in ("sem-inc", "sem-add-imm"):
                    cums = lane_orders.setdefault(u.id, [])
                    prev = cums[-1][0] if cums else 0
                    cums.append((prev + u.update_value, i))
    for i in insts:
        si = i.sync_info
        if si is None or type(i).__name__ != "InstDMACopy":
            continue
        if len(si.on_wait) <= 1 or _out_name(i) != "y":
            continue
        kept = []
        for w in si.on_wait:
            pub = None
            for cum, d in lane_orders.get(w.id, ()):
                if cum >= (w.wait_value or 0):
                    pub = d
                    break
            if pub is not None and _out_name(pub) == "y":
                continue
            kept.append(w)
        if len(kept) != len(si.on_wait):
            i.sync_info = mybir.SyncInfo(on_wait=kept, on_update=list(si.on_update))


_NC_CACHE = None


def _get_nc():
    global _NC_CACHE
    if _NC_CACHE is None:
        _NC_CACHE = build_bass()
    return _NC_CACHE


def make_in_maps(x, w1, w3, w2):
    """Shard + fp16-cast + pre-permute the full inputs for the 8 cores."""
    x16 = np.asarray(x, dtype=np.float32).astype(NPF16)
    w116 = np.asarray(w1, dtype=np.float32).astype(NPF16)
    w316 = np.asarray(w3, dtype=np.float32).astype(NPF16)
    w216 = np.asarray(w2, dtype=np.float32).astype(NPF16)
    ident = np.eye(SEG, dtype=NPF16)

    in_maps = []
    for c in range(N_CORES):
        es = slice(c * EPC, (c + 1) * EPC)
        xs = x16[c * TPC : (c + 1) * TPC]  # (TPC, D)
        # xt[p, k, t] = x[t, p*KT + k]
        xt = np.ascontiguousarray(xs.reshape(TPC, 128, KT).transpose(1, 2, 0))
        # w1/w3: (EPC, F, D) -> [e, p, k, f] = w[f, p*KT + k], split f halves
        w1c = np.ascontiguousarray(
            w116[es]
            .transpose(0, 2, 1)
            .reshape(EPC, 128, KT, 2, FB)
            .transpose(0, 3, 1, 2, 4)
        )
        w3c = np.ascontiguousarray(
            w316[es]
            .transpose(0, 2, 1)
            .reshape(EPC, 128, KT, 2, FB)
            .transpose(0, 3, 1, 2, 4)
        )
        # w2: (EPC, D, F) -> [e, p, k, d] = w2[d, k*128 + p] (f = k*128+p
        # to match the PE-transpose layout of hT), split d halves
        w2c = np.ascontiguousarray(
            w216[es]
            .transpose(0, 2, 1)
            .reshape(EPC, KT, 128, 2, FB)
            .transpose(0, 3, 2, 1, 4)
        )
        in_maps.append(
            {"xt": xt, "w1t": w1c, "w3t": w3c, "w2t": w2c, "ident": ident}
        )
    return in_maps


def kernel(x, w1, w3, w2, expert_ids, seg_starts, seg_ends):
    w1 = np.asarray(w1, dtype=np.float32)
    w3 = np.asarray(w3, dtype=np.float32)
    w2 = np.asarray(w2, dtype=np.float32)
    eid = np.asarray(expert_ids).astype(np.int64)

    # reference: segment s (tokens [s*SEG, (s+1)*SEG)) uses expert_ids[s]
    if not np.array_equal(eid, np.arange(E)):
        w1, w3, w2 = w1[eid], w3[eid], w2[eid]

    in_maps = make_in_maps(x, w1, w3, w2)
    nc = _get_nc()
    res = run_bass_kernel_spmd(nc, in_maps, core_ids=list(range(N_CORES)))
    out = np.concatenate([r["y"] for r in res.results], axis=0)
    return out.astype(np.float32)
